# revision 44
# baseline (speedup 1.0000x reference)
"""Trainium2 Bass kernel for nn_AttentionHead (B=4, n_ctx=4096, d_model=1024,
d_hidden=64, causal, scale=1/sqrt(d_model)).

Sharding: 8 cores = 4 batches x 2 balanced causal shards. Core (b, s) handles
the 2048 query rows in 64-row chunks with chunk%2 == s. Keys/x-columns are
permuted per core (my-parity chunks first within each 512-key ntile) so that
every core runs the IDENTICAL SPMD program:

  - slot j (0..3) = 512 queries = my chunks of ntiles 2j, 2j+1
  - slot j attends k-tiles t = 0..8(j+1)-1 (128 permuted keys each)
  - k-tiles t < 8j are fully open; t = 8j + r (r in 0..7) get an additive
    causal mask that depends only on (r, s) -> 8 mask tiles per core, sent
    as data.

v2 design notes (cost model: matmul cost = moving-dim rows only; DMA
transfers from different issuing engines overlap; same-engine serialize;
GPSIMD cannot touch PSUM on real HW):

  A (v4): x ships as fp8 hi + fp8 residual lo (same bytes as bf16);
     weights ship as 32x-prescaled fp8 hi + residual lo. KT/VT/Q run as
     fp8 DoubleRow chains at 0.5 cycles/row, with the cross-term
     correction chains (w8^T@xl + wl^T@x8) accumulating into the SAME
     PSUM group as the hi chain (residuals are encoded at matching
     scale), giving ~0.2% projections -- better than bf16 at a quarter
     of the PE cost. kvt bf16 holds 32*[K;V]; the 32x folds into the
     exp scale and wobo. bk is dropped (a per-query score shift,
     softmax-invariant), bov folds into the wobo bias row host-side as
     bo + Wo@bov, bq applies in the Q evac. V transposed to natural
     [k,64] bf16 layout via PE transpose; v65 keeps an appended ones
     column (DVE memset) so E^T@[V|1] also yields the softmax
     denominator.
  B (v3): S^T[k,q] for a tile is ONE fp8e4 DoubleRow matmul at 0.5
     cycles/row computing K^T@Q + U^T@Vm in a single PSUM group: subtile
     0 = [K;V-junk] fp8 against Q fp8 (rows 64:128 of the Q operand are
     zero so the junk V rows contribute nothing), subtile 1 = the causal
     mask factored as U^T@Vm (U in {0,2} one-hot per dead-prefix
     threshold, Vm rows = -240 prefixes -> additive -480, exp -> ~3e-7);
     open tiles point subtile 1 at a zero block. Masked tiles are paired
     (8j,8j+2),(8j+1,8j+3),(8j+4,8j+6),(8j+5,8j+7) so both pair halves
     share one causal offset and a single strided exp covers the pair;
     offsets are 128-aligned dead prefixes (0/128/0/128/256/384/256/384).
     K fp8 copies are made by the otherwise-idle Pool engine from the
     bf16 kvt evacuations; Q is evacuated straight to fp8 by DVE.
  C (transposed vs v1): O[q,65] += E_chunk^T @ V65_tile, i.e. E is the
     stationary operand and the 65-wide V65 is moving: 65 rows/chunk-tile
     instead of 512/tile -- less than half the PE cost of v1's C. Col 64
     accumulates the denominator per q-partition. O lives in two PSUM
     banks (chunks 0-1 / 2-3) because group state is bank-granular and a
     bank is only readable once its group stops -- the split lets the
     first chunks finish while the rest still accumulate.
  D: per 128-q chunk: recip = 1/O[:,64] (DVE), normalize-copy
     O*recip -> bf16 (fused into the mandatory PSUM evacuation; makes
     col 64 exactly 1.0), PE-transpose to OT[65,128], DVE-copy to SBUF,
     then y = OT^T @ [Wo^T; bo] -- the 1.0 row adds bo exactly, so no
     per-element recip multiply is needed after the matmul. y is
     evacuated to bf16 on DVE (tail: ACT Identity + DVE halves; identity
     shares exp's table set) and stored per-slot (tail: per-chunk).

Schedule: all x ntiles are prefetched up front (x0 in quarters + evens
on the SP DMA stream, x1 + odds shared with ACT/SP; masks + wobo on the
otherwise-idle Pool stream), so the A stages can be pulled far forward.
Slot pairs are spread between A stages (s0 around A2, s1 around A4/A5,
s2 around A6/A7) so the ACT exp stream -- the binding engine late in
the kernel -- is fed continuously and slot 3's 16 pairs start as early
as the data allows. Finished slots' OT/D/store work is popped one unit
per pair into later slots' pair loops. y is written bf16 and upcast on
host (~0.2% fro error vs the 2e-2 budget). A few warmup matmuls on the
tiny blobw keep the PE pipeline primed during the DMA fill.
"""

import math

import numpy as np

D = 1024
H = 64
N = 4096
B = 4
CH = 64  # query chunk size (rows)
NT = 8  # ntiles of 512 keys
NEG = -1e10

_PROG = None  # cached compiled program
_META = None  # cached mask offsets/ends


# ---------------------------------------------------------------- host layout


def _key_order(s: int) -> np.ndarray:
    order = []
    for n in range(NT):
        mine = [8 * n + t for t in range(8) if t % 2 == s]
        theirs = [8 * n + t for t in range(8) if t % 2 != s]
        for c in mine + theirs:
            order.extend(range(CH * c, CH * c + CH))
    return np.array(order)


def _masks(s: int) -> np.ndarray:
    ko = _key_order(s)
    qo = np.array([CH * c + i for c in range(s, 64, 2) for i in range(CH)])
    m = np.zeros((8, 128, 512), dtype=np.float32)
    for r in range(8):
        keys = ko[128 * r : 128 * (r + 1)]
        qs = qo[0:512]
        m[r] = np.where(keys[:, None] <= qs[None, :], 0.0, NEG)
    return m


def _mask_meta():
    """Per masked-tile r (min/max over both parities so the shared program is
    valid for either): OFF_C = 128-floored fully-dead q-prefix, END = end of
    the nonzero mask band."""
    global _META
    if _META is not None:
        return _META
    offs = []
    ends = []
    ms = [_masks(0), _masks(1)]
    for r in range(8):
        offr, endr = [], []
        for s in (0, 1):
            dead = ms[s][r] != 0.0
            colall = dead.all(axis=0)
            off = 0
            while off < 512 and colall[off]:
                off += 1
            anyd = dead.any(axis=1).any()
            cols = np.nonzero(dead.any(axis=0))[0]
            end = int(cols.max()) + 1 if cols.size else 0
            offr.append(off)
            endr.append(end)
        offs.append((min(offr) // 128) * 128)
        ends.append(max(endr))
    _META = (offs, ends)
    return _META


FP8_MASK = -240.0  # e4m3 max-magnitude step; U entries are 64 -> -15360,
# which under the 1/1024 exp scale gives exp(-15) ~ 3e-7 for dead keys


def _umask_qmz(s: int):
    """Factor each diagonal tile's additive mask as U_r^T @ Vm_r (both fp8):
    U_r[i, k] = 2 iff key k's dead q-prefix length == thr_i, Vm_r[i, q] =
    FP8_MASK for q < thr_i. Verified exact against _masks."""
    m = _masks(s)
    U = np.zeros((128, 8 * 128), dtype=np.float32)
    Vm = np.zeros((128, 8 * 512), dtype=np.float32)
    for r in range(8):
        dead = m[r] != 0.0
        d = dead.sum(axis=1)
        thrs = sorted(set(d.tolist()) - {0})
        assert len(thrs) <= 128, (s, r, len(thrs))
        for i, thr in enumerate(thrs):
            U[i, 128 * r + np.nonzero(d == thr)[0]] = 64.0
            Vm[i, 512 * r : 512 * r + thr] = FP8_MASK
        got = U[:, 128 * r : 128 * (r + 1)].T @ Vm[:, 512 * r : 512 * (r + 1)]
        want = np.where(dead, 64 * FP8_MASK, 0.0)
        assert np.array_equal(got, want), (s, r)
    return U, Vm


# ---------------------------------------------------------------- bass program

# blobw bf16-column layout: [idb 128 | bias(f32) 4]  (tiny, lands first so
# the PE warmup can start during the x0 fill; bias col 0 = bq)
_BW_IDB = 0
_BW_BIAS = 128
_BW_COLS = 132
# blob1 fp8-column layout: [wkv_lo 8x128 | wkv_hi 8x128 | wq_lo 8x64 |
# wq_hi 8x64] -- lo blocks lead hi blocks so a +8-block step-slice pairs
# (lo_w, hi_w) against x's (hi_x, lo_x) for the cross-term corr chains.
# wkv is 32x-prescaled (wkv' = 32*[Wk;Wov].T, lo = fp8(32*(wkv' -
# fp8(wkv')))); wq_hi = fp8(Wq.T) unscaled with wq_lo = fp8(32*residual)
# catching its subnormal error, so the Q combine's (corr/32 + hi) yields
# Q directly.
_B1_WKV_LO = 0
_B1_WKV_HI = 1024
_B1_WQ_LO = 2048
_B1_WQ_HI = 2560
_B1_COLS = 3072


def _build():
    import concourse.mybir as mybir
    import concourse.tile as tile
    from concourse import bacc

    f32 = mybir.dt.float32
    bf16 = mybir.dt.bfloat16
    fp8 = mybir.dt.float8e4

    OFF_C, ENDS = _mask_meta()

    nc = bacc.Bacc("TRN2", target_bir_lowering=False, debug=False, num_devices=8)

    xh = nc.dram_tensor("xh", [NT, 128, 16, 512], fp8, kind="ExternalInput").ap()
    blobw = nc.dram_tensor("blobw", [128, _BW_COLS], bf16, kind="ExternalInput").ap()
    blob1 = nc.dram_tensor("blob1", [128, _B1_COLS], fp8, kind="ExternalInput").ap()
    umask = nc.dram_tensor("umask", [128, 8 * 128], fp8, kind="ExternalInput").ap()
    qmzc = nc.dram_tensor("qmzc", [128, 512 + 8 * 512], fp8, kind="ExternalInput").ap()
    qz = nc.dram_tensor("qz", [64, 2048], fp8, kind="ExternalInput").ap()
    wobo = nc.dram_tensor("wobo", [65, 1024], bf16, kind="ExternalInput").ap()
    y = nc.dram_tensor("y", [2048, 1024], bf16, kind="ExternalOutput").ap()

    Exp = mybir.ActivationFunctionType.Exp
    Identity = mybir.ActivationFunctionType.Identity
    DoubleRow = mybir.MatmulPerfMode.DoubleRow
    mult = mybir.AluOpType.mult
    add_op = mybir.AluOpType.add
    # kvt holds 32*[K;V] (prescaled weights), so S' = 32*S and the 32 folds
    # into the exp scale; wobo is divided by 32 host-side.
    scale = 1.0 / (32.0 * math.sqrt(D))

    with tile.TileContext(nc) as tc:
        with (
            tc.tile_pool(name="consts", bufs=1) as consts,
            tc.tile_pool(name="xp", bufs=8) as xpool,
            tc.tile_pool(name="ep", bufs=9) as epool,
            tc.tile_pool(name="osp", bufs=4) as ospool,
            tc.tile_pool(name="yb", bufs=3) as ypool,
            tc.tile_pool(name="pkv", bufs=1, space="PSUM") as pkv,
            tc.tile_pool(name="pq", bufs=1, space="PSUM") as pq,
            tc.tile_pool(name="po", bufs=1, space="PSUM") as po,
            tc.tile_pool(name="ps", bufs=2, space="PSUM") as ps,
        ):
            # ---- constants. Tiny blobw leads the SP stream (PE warmup
            # dependency); blob1 rides the ACT stream concurrently with x0.
            blobw_sb = consts.tile([128, _BW_COLS], bf16)
            nc.sync.dma_start(blobw_sb[:], blobw[:])
            blob1_sb = consts.tile([128, _B1_COLS], fp8)
            nc.sync.dma_start(blob1_sb[:], blob1[:])
            # 128-col blocks: 0..7 = wkv_lo chunks, 8..15 = wkv_hi chunks
            wb128 = blob1_sb[:].rearrange("p (b c) -> p b c", c=128)
            # 64-col blocks: 32..39 = wq_lo chunks, 40..47 = wq_hi chunks
            wb64 = blob1_sb[:].rearrange("p (b c) -> p b c", c=64)
            idb_v = blobw_sb[:, _BW_IDB : _BW_IDB + 128]
            bias_v = blobw_sb[:, _BW_BIAS : _BW_BIAS + 4].bitcast(f32)

            kvt_sb = consts.tile([128, N], bf16)  # rows 0:64 KT, 64:128 VT
            # [U8 8x128 | kvt8 32x128]: fp8 stationary blocks for the fused
            # score+mask DoubleRow matmuls (kvt8 = Pool-copied [K;V-junk])
            kmask_sb = consts.tile([128, 1024 + 4096], fp8)
            # [zeros 512 | Vm8 8x512 | qt8 4x512]: fp8 moving blocks; qt8
            # rows 64:128 are zero (DMA'd) so kvt8's V rows never contribute
            qmz_sb = consts.tile([128, 512 + 4096 + 2048], fp8)
            # slot 0's queries in bf16: its rows attend few keys, so fp8
            # score noise doesn't average out -- slot 0 runs bf16 scores
            qt0_sb = consts.tile([H, 512], bf16)
            v65_sb = consts.tile([128, 32 * 65], bf16)
            nc.vector.memset(
                v65_sb[:].rearrange("p (t c) -> p t c", c=65)[:, :, 64:65], 1.0
            )
            wobo_sb = consts.tile([65, 1024], bf16)
            ot_sb = consts.tile([65, 2048], bf16)
            recip_sb = consts.tile([128, 16], f32)
            scratch_sb = consts.tile([1, 8], f32)

            # ---- prefetch ALL x ntiles up front over the two HWDGE DMA
            # streams (even ntiles + x0-quarters on SP, odd on ACT): x7 lands
            # ~15us in, so the A stages can be pulled far forward, unlocking
            # slot 3's exp work early enough that ACT never gates the end.
            # x tiles: [hi 8x512 | lo 8x512] fp8 per ntile. All on the SP
            # stream: DMA transfers serialize on the global DMA-engine
            # resource regardless of issuing engine, so landing order is
            # issue order; keeping ACT's SEQ free of DMA issue holds.
            xns = [
                xpool.tile([128, 8192], fp8, tag="x", name=f"xn{n}")
                for n in range(NT)
            ]
            xv0 = xns[0][:].rearrange("p (b c) -> p b c", c=1024)
            xd0 = (
                xh[0]
                .rearrange("p c f -> p (c f)")
                .rearrange("p (b c) -> p b c", c=1024)
            )
            for qq in range(4):
                # quarter q = [hi pair q | lo pair q] so chain1 cp=q and
                # corr chunks 2q,2q+1 unblock together
                nc.sync.dma_start(xv0[:, qq : qq + 5 : 4], xd0[:, qq : qq + 5 : 4])
            for n in range(1, NT):
                nc.sync.dma_start(xns[n][:], xh[n].rearrange("p c f -> p (c f)"))

            # prewarm the ACT exp table while DMAs stream
            nc.scalar.activation(
                scratch_sb[:], idb_v[0:1, 0:8], Exp, bias=0.0, scale=1.0
            )

            # mask factors + wobo ride the Pool DMA stream, which is
            # otherwise idle until the kvt8 copies -- they land well before
            # slot 0 needs them and keep the HWDGE streams clear for x.
            nc.gpsimd.dma_start(kmask_sb[:, 0:1024], umask[:])
            nc.gpsimd.dma_start(qmz_sb[:, 0 : 512 + 4096], qmzc[:])
            nc.gpsimd.dma_start(qmz_sb[64:128, 4608 : 4608 + 2048], qz[:])
            nc.gpsimd.dma_start(wobo_sb[:], wobo[:])

            # PE pipeline/p-state warmup on blobw during the x0 fill
            warm = ps.tile([128, 1024], f32, tag="s")
            for _ in range(3):
                nc.tensor.matmul(
                    warm[:, 0:128], idb_v[:], idb_v[:], start=True, stop=True
                )

            # ---------------- stage A: projections for ntile n (fp8
            # DoubleRow chains; kvt = main + corr/32 holds 32*[K;V])
            def emit_a(n):
                xv = xns[n][:].rearrange("p (b c) -> p b c", c=512)
                kvp = pkv.tile([128, 512], f32, tag="kv")
                for cp in range(4):
                    for p in range(2):
                        nc.tensor.matmul(
                            kvp[:, 256 * p : 256 * (p + 1)],
                            wb128[:, 8 + 2 * cp : 8 + 2 * cp + 2, :],
                            xv[:, 2 * cp : 2 * cp + 2, 256 * p : 256 * (p + 1)],
                            start=(cp == 0 and p == 0),
                            stop=False,
                            perf_mode=DoubleRow,
                        )
                # cross-term correction chains accumulate straight into the
                # main group (residuals are encoded at matching scale)
                for c in range(8):
                    for p in range(2):
                        nc.tensor.matmul(
                            kvp[:, 256 * p : 256 * (p + 1)],
                            wb128[:, c : c + 9 : 8, :],
                            xv[:, c : c + 9 : 8, 256 * p : 256 * (p + 1)],
                            start=False,
                            stop=(c == 7 and p == 1),
                            perf_mode=DoubleRow,
                        )
                nc.vector.tensor_copy(
                    kvt_sb[:, 512 * n : 512 * (n + 1)], kvp[:]
                )
                # fp8 copy of [K;V-junk] for the DoubleRow score matmuls on
                # the otherwise-idle Pool engine (V rows harmless: the Q
                # operand's rows 64:128 are zero)
                nc.gpsimd.tensor_copy(
                    kmask_sb[:, 1024 + 512 * n : 1024 + 512 * (n + 1)],
                    kvt_sb[:, 512 * n : 512 * (n + 1)],
                )
                # Q: hi + cross chains in one PSUM group; evac divides the
                # 32x weight prescale back out and adds bq
                qp = pq.tile([64, 256], f32, tag="q", name="qp")
                for cp in range(4):
                    nc.tensor.matmul(
                        qp[:],
                        wb64[:, 40 + 2 * cp : 40 + 2 * cp + 2, :],
                        xv[:, 2 * cp : 2 * cp + 2, 0:256],
                        start=(cp == 0),
                        stop=False,
                        perf_mode=DoubleRow,
                    )
                for c in range(8):
                    nc.tensor.matmul(
                        qp[:],
                        wb64[:, 32 + c : 32 + c + 9 : 8, :],
                        xv[:, c : c + 9 : 8, 0:256],
                        start=False,
                        stop=(c == 7),
                        perf_mode=DoubleRow,
                    )
                qdst = (
                    qt0_sb[:, 256 * n : 256 * (n + 1)]
                    if n < 2
                    else qmz_sb[0:64, 4608 + 256 * n : 4608 + 256 * (n + 1)]
                )
                nc.vector.tensor_scalar(
                    out=qdst,
                    in0=qp[:],
                    scalar1=1.0 / 32.0,
                    scalar2=bias_v[0:64, 0:1],
                    op0=mult,
                    op1=add_op,
                )
                vp = pkv.tile([128, 256], bf16, tag="kv")
                for i, t in enumerate(range(4 * n, 4 * n + 4)):
                    nc.tensor.transpose(
                        vp[:, 64 * i : 64 * (i + 1)],
                        kvt_sb[64:128, 128 * t : 128 * (t + 1)],
                        idb_v[64:128, 64:128],
                    )
                nc.vector.tensor_copy(
                    v65_sb[:].rearrange("p (t c) -> p t c", c=65)[
                        :, 4 * n : 4 * n + 4, 0:64
                    ],
                    vp[:].rearrange("p (t c) -> p t c", c=64),
                )

            # ---------------- per-chunk finish in two phases so the Pool
            # ot copy of chunk c overlaps PE work of the previous phase-2:
            #   phase 1: OT transpose (PE) + ot_sb copy (Pool)
            #   phase 2: D matmuls (PE) + y copy (Pool) + per-slot store
            phase1 = []
            phase2 = []
            ys_done = {}  # slot -> chunks copied (store fires on the 4th)

            def emit_phase1(task):
                j, c, osb, ys = task
                i = 4 * j + c
                pot = pq.tile([65, 128], bf16, tag="q")
                nc.tensor.transpose(pot[:], osb[:], idb_v[:])
                nc.vector.tensor_copy(ot_sb[:, 128 * i : 128 * (i + 1)], pot[:])
                phase2.append(task)

            def emit_phase2(task, split_store=False):
                j, c, _, ys = task
                i = 4 * j + c
                if split_store:
                    # tail: halves in separate PSUM pools (kv is free after
                    # the A stages) so D matmuls don't wait on earlier
                    # chunks' y copies; copies alternate Pool/DVE; stores
                    # alternate over the idle SP/ACT DMA streams
                    for d in range(2):
                        # d0 alternates the freed kv/q banks so consecutive
                        # chunks' D matmuls aren't paced by copy round-trips
                        if d == 0:
                            tag0 = "kv" if c % 2 == 0 else "q"
                            yph = (pkv if c % 2 == 0 else pq).tile(
                                [128, 512], f32, tag=tag0, name="yph0"
                            )[:, 0:512]
                        else:
                            yph = ps.tile([128, 1024], f32, tag="s", name="yph1")[
                                :, 0:512
                            ]
                        nc.tensor.matmul(
                            yph[:],
                            ot_sb[:, 128 * i : 128 * (i + 1)],
                            wobo_sb[:, 512 * d : 512 * (d + 1)],
                            start=True,
                            stop=True,
                        )
                        if d == 0:
                            # ACT is idle after its last exp; identity shares
                            # exp's table set so no table reload
                            nc.scalar.activation(
                                ys[:, 1024 * c : 1024 * c + 512],
                                yph[:],
                                Identity,
                                bias=0.0,
                                scale=1.0,
                            )
                        else:
                            nc.vector.tensor_copy(
                                ys[:, 1024 * c + 512 : 1024 * (c + 1)], yph[:]
                            )
                    if c == 3:
                        # final chunk: store each half right behind its copy
                        # so the kernel-ending store chain is shorter
                        for d in range(2):
                            seng = nc.sync if d == 0 else nc.scalar
                            seng.dma_start(
                                y[512 * j + 128 * c : 512 * j + 128 * (c + 1), :]
                                .rearrange("p (d f) -> p d f", d=2)[:, d],
                                ys[:, 1024 * c + 512 * d : 1024 * c + 512 * (d + 1)],
                            )
                    else:
                        # Pool SWDGE: its engine and DMA stream are idle in
                        # the drain, keeping ACT free for the final chunk's
                        # identity-copy and SP free for the final half-stores
                        nc.gpsimd.dma_start(
                            y[512 * j + 128 * c : 512 * j + 128 * (c + 1), :],
                            ys[:, 1024 * c : 1024 * (c + 1)],
                        )
                elif j >= 2:
                    # popped during slot 3 (post-A7): the kv/q banks are
                    # free -- using them keeps y evacuation round-trips out
                    # of the "s" rotation that feeds ACT's exp stream
                    for d in range(2):
                        pool, tag = ((pkv, "kv"), (pq, "q"))[(c + d) % 2]
                        yph = pool.tile([128, 512], f32, tag=tag, name="ypl")[
                            :, 0:512
                        ]
                        nc.tensor.matmul(
                            yph[:],
                            ot_sb[:, 128 * i : 128 * (i + 1)],
                            wobo_sb[:, 512 * d : 512 * (d + 1)],
                            start=True,
                            stop=True,
                        )
                        nc.vector.tensor_copy(
                            ys[:, 1024 * c + 512 * d : 1024 * c + 512 * (d + 1)],
                            yph[:],
                        )
                    ys_done[j] = ys_done.get(j, 0) + 1
                    if ys_done[j] == 4:
                        nc.sync.dma_start(
                            y[512 * j : 512 * (j + 1), :].rearrange(
                                "(t p) d -> p t d", p=128
                            ),
                            ys[:].rearrange("p (t d) -> p t d", t=4),
                        )
                else:
                    yp = ps.tile([128, 1024], f32, tag="s")
                    for d in range(2):
                        nc.tensor.matmul(
                            yp[:, 512 * d : 512 * (d + 1)],
                            ot_sb[:, 128 * i : 128 * (i + 1)],
                            wobo_sb[:, 512 * d : 512 * (d + 1)],
                            start=True,
                            stop=True,
                        )
                    nc.vector.tensor_copy(ys[:, 1024 * c : 1024 * (c + 1)], yp[:])
                    # stores ride the SP DMA stream, idle after the x loads
                    ys_done[j] = ys_done.get(j, 0) + 1
                    if ys_done[j] == 4:
                        nc.sync.dma_start(
                            y[512 * j : 512 * (j + 1), :].rearrange(
                                "(t p) d -> p t d", p=128
                            ),
                            ys[:].rearrange("p (t d) -> p t d", t=4),
                        )

            popflip = [False]

            def pop_pending():
                # alternate phases so consecutive pops don't serialize on
                # the single pot slot's DVE round-trip
                popflip[0] = not popflip[0]
                if phase1 and (popflip[0] or not phase2):
                    emit_phase1(phase1.pop(0))
                elif phase2:
                    emit_phase2(phase2.pop(0))

            # ---------------- stage B + C for slot j, steppable by pair so
            # slots can interleave with A stages and each other
            class Slot:
                def __init__(self, j):
                    self.j = j
                    # two accumulator banks (chunks 0-1 / 2-3): PSUM group
                    # state is bank-granular, and a bank can only be read
                    # once its group stops -- the split lets chunks 0-1
                    # finish while 2-3 still accumulate
                    self.o_a = po.tile([128, 2 * 65], f32, tag="oA", name=f"oa{j}")
                    self.o_b = po.tile([128, 2 * 65], f32, tag="oB", name=f"ob{j}")
                    self.ys = ypool.tile([128, 4096], bf16, tag="ys", name=f"ys{j}")
                    # open tiles pair consecutively; masked tiles are paired
                    # so both halves share the same causal offset, letting a
                    # single strided exp cover the pair
                    self.pairs = [(t, t + 1) for t in range(0, 8 * j, 2)] + [
                        (8 * j + a, 8 * j + b)
                        for a, b in ((0, 2), (1, 3), (4, 6), (5, 7))
                    ]
                    # PSUM accumulation groups are bank-granular (2KB): all
                    # four 65-col chunk accumulators share one bank, so
                    # start/stop go on the first/last live matmul in emission
                    # order (start's pending-zero covers the full bank).
                    # per-bank group bounds (first/last live matmul in
                    # emission order) + the pair at which each bank closes
                    self.first_tc = {}
                    self.last_tc = {}
                    self.bank_pair = {}
                    for b, cs in ((0, (0, 1)), (1, (2, 3))):
                        lv = [
                            (t, c)
                            for pr in self.pairs
                            for t in pr
                            for c in cs
                            if self.live(t, c)
                        ]
                        self.first_tc[b] = lv[0]
                        self.last_tc[b] = lv[-1]
                        self.bank_pair[b] = max(
                            k
                            for k, pr in enumerate(self.pairs)
                            if any(self.live(t, c) for t in pr for c in cs)
                        )
                    self.pi = 0
                    self.prevs = []

                def obank(self, c):
                    return (self.o_a if c < 2 else self.o_b)[
                        :, 65 * (c % 2) : 65 * (c % 2) + 65
                    ]

                def live(self, t, c):
                    rr = t - 8 * self.j
                    return rr < 0 or OFF_C[rr] < 128 * (c + 1)

                def emit_ct(self, k, pr, et, offs):
                    for h in range(2):
                        t = pr[h]
                        for c in range(4):
                            if not self.live(t, c):
                                continue  # chunk fully causally dead
                            b = c // 2
                            nc.tensor.matmul(
                                self.obank(c),
                                et[:, 512 * h + 128 * c : 512 * h + 128 * (c + 1)],
                                v65_sb[:, 65 * t : 65 * (t + 1)],
                                start=((t, c) == self.first_tc[b]),
                                stop=((t, c) == self.last_tc[b]),
                            )
                    for b in (0, 1):
                        if self.bank_pair[b] == k:
                            self.finish_chunk(2 * b)
                            self.finish_chunk(2 * b + 1)

                def finish_chunk(self, c):
                    # denominator + normalize/evacuate O chunk to bf16
                    i = 4 * self.j + c
                    ob = self.obank(c)
                    nc.vector.reciprocal(recip_sb[:, i : i + 1], ob[:, 64:65])
                    osb = ospool.tile([128, 65], bf16, tag="osb")
                    nc.vector.tensor_scalar(
                        out=osb[:],
                        in0=ob[:],
                        scalar1=recip_sb[:, i : i + 1],
                        scalar2=None,
                        op0=mult,
                    )
                    phase1.append((self.j, c, osb, self.ys))

                def step(self, npairs, pops=True):
                    j = self.j
                    kb = kmask_sb[:].rearrange("p (b c) -> p b c", c=128)
                    qb = qmz_sb[:].rearrange("p (b c) -> p b c", c=512)
                    for pr in self.pairs[self.pi : self.pi + npairs]:
                        if pops:
                            pop_pending()
                        sp = ps.tile([128, 1024], f32, tag="s")
                        offs = []
                        for h in range(2):
                            t = pr[h]
                            rr = t - 8 * j
                            off = 0 if rr < 0 else OFF_C[rr]
                            offs.append(off)
                            pieces = (
                                [(off, 512)]
                                if 512 - off <= 256
                                else [(off, 256), (256, 512)]
                            )
                            if j == 0:
                                # slot 0: few live keys per query -> fp8
                                # score noise doesn't average out; bf16
                                # scores + plain-fp8 mask product
                                nc.tensor.matmul(
                                    sp[:, 512 * h + off : 512 * (h + 1)],
                                    kvt_sb[0:64, 128 * t : 128 * (t + 1)],
                                    qt0_sb[:, off:512],
                                    start=True,
                                    stop=False,
                                )
                                nc.tensor.matmul(
                                    sp[:, 512 * h + off : 512 * (h + 1)],
                                    kmask_sb[:, 128 * rr : 128 * (rr + 1)],
                                    qmz_sb[:, 512 * (1 + rr) + off : 512 * (2 + rr)],
                                    start=False,
                                    stop=True,
                                )
                                continue
                            # fused scores+mask: one fp8 DoubleRow group,
                            # split into <=256-col pieces (moving limit 512)
                            b0k = rr if rr >= 0 else 0
                            dk = 8 + t - b0k
                            b0q = 1 + rr if rr >= 0 else 0
                            dq = 9 + j - b0q
                            lhsT = kb[:, b0k : b0k + dk + 1 : dk, :]
                            for pc, (o0, o1) in enumerate(pieces):
                                nc.tensor.matmul(
                                    sp[:, 512 * h + o0 : 512 * h + o1],
                                    lhsT,
                                    qb[:, b0q : b0q + dq + 1 : dq, o0:o1],
                                    start=(pc == 0),
                                    stop=(pc == len(pieces) - 1),
                                    perf_mode=DoubleRow,
                                )
                        et = epool.tile([128, 1024], bf16, tag="e")
                        if offs == [0, 0]:
                            nc.scalar.activation(
                                et[:], sp[:], Exp, bias=0.0, scale=scale
                            )
                        elif offs[0] == offs[1]:
                            o = offs[0]
                            nc.scalar.activation(
                                et[:].rearrange("p (b f) -> p b f", b=2)[:, :, o:512],
                                sp[:].rearrange("p (b f) -> p b f", b=2)[:, :, o:512],
                                Exp,
                                bias=0.0,
                                scale=scale,
                            )
                        else:
                            for h in range(2):
                                o = 512 * h + offs[h]
                                nc.scalar.activation(
                                    et[:, o : 512 * (h + 1)],
                                    sp[:, o : 512 * (h + 1)],
                                    Exp,
                                    bias=0.0,
                                    scale=scale,
                                )
                        self.prevs.append((self.pi, pr, et, offs))
                        if len(self.prevs) > 2:
                            self.emit_ct(*self.prevs.pop(0))
                        self.pi += 1

                def finish(self):
                    for p in self.prevs:
                        self.emit_ct(*p)
                    self.prevs = []

            # ---------------- master schedule: A stages pulled forward (x is
            # prefetched), slot pairs spread between them so ACT's exp stream
            # is fed continuously from ~10us on and slot 3 unlocks early.
            emit_a(0)
            emit_a(1)
            s0 = Slot(0)
            s0.step(2)
            emit_a(2)
            s0.step(2)
            s0.finish()
            emit_a(3)
            s1 = Slot(1)
            s1.step(3)
            emit_a(4)
            s1.step(3)
            emit_a(5)
            s1.step(2)
            s1.finish()
            s2 = Slot(2)
            s2.step(4)
            emit_a(6)
            s2.step(5)
            emit_a(7)
            s2.step(3)
            s2.finish()
            s3 = Slot(3)
            s3.step(16)
            s3.finish()

            # tail: drain slot 3 with per-chunk stores so the final store is
            # a quarter-slot, shortening the serial tail
            for task in phase1:
                emit_phase1(task)
            phase1 = []
            for task in phase2:
                emit_phase2(task, split_store=True)

    nc.compile()
    return nc


def _get_prog():
    global _PROG
    if _PROG is None:
        _PROG = _build()
    return _PROG


# ---------------------------------------------------------------- host inputs


def _xh(xb, korder):
    """[ntile, partition, 16, 512] fp8 hi|lo layout of x[b][korder].T."""
    import ml_dtypes

    fp8 = ml_dtypes.float8_e4m3
    xt = np.ascontiguousarray(xb[korder].T)  # [1024, 4096] f32
    hi = xt.astype(fp8)
    lo = (xt - hi.astype(np.float32)).astype(fp8)

    def lay(a):
        return a.reshape(8, 128, 8, 512).transpose(2, 1, 0, 3)

    return np.ascontiguousarray(np.concatenate([lay(hi), lay(lo)], axis=2))


def _blobw(bq):
    import ml_dtypes

    blob = np.zeros((128, _BW_COLS), dtype=ml_dtypes.bfloat16)
    blob[:, _BW_IDB : _BW_IDB + 128] = np.eye(128, dtype=ml_dtypes.bfloat16)
    biases = np.zeros((128, 2), dtype=np.float32)
    biases[0:64, 0] = bq
    blob[:, _BW_BIAS : _BW_BIAS + 4] = biases.view(np.uint16).view(ml_dtypes.bfloat16)
    return blob


def _blob1(Wq, Wk, Wov):
    import ml_dtypes

    fp8 = ml_dtypes.float8_e4m3
    blob = np.zeros((128, _B1_COLS), dtype=fp8)
    wkv = 32.0 * np.concatenate([Wk, Wov], axis=0).T.astype(np.float32)  # [1024,128]
    hi = wkv.astype(fp8)
    lo = (wkv - hi.astype(np.float32)).astype(fp8)

    def lay128(a):
        return a.reshape(8, 128, 128).transpose(1, 0, 2).reshape(128, 1024)

    blob[:, _B1_WKV_LO : _B1_WKV_LO + 1024] = lay128(lo)
    blob[:, _B1_WKV_HI : _B1_WKV_HI + 1024] = lay128(hi)
    wq = 32.0 * Wq.T.astype(np.float32)  # [1024, 64], same 32x prescale
    qhi = wq.astype(fp8)
    qlo = (wq - qhi.astype(np.float32)).astype(fp8)

    def lay64(a):
        return a.reshape(8, 128, 64).transpose(1, 0, 2).reshape(128, 512)

    blob[:, _B1_WQ_LO : _B1_WQ_LO + 512] = lay64(qlo)
    blob[:, _B1_WQ_HI : _B1_WQ_HI + 512] = lay64(qhi)
    return blob


def _in_map(x, Wq, bq, Wk, bk, Wov, bov, Wo, bo, core):
    import ml_dtypes

    fp8 = ml_dtypes.float8_e4m3
    b, s = divmod(core, 2)
    U, Vm = _umask_qmz(s)
    qmzc = np.zeros((128, 512 + 8 * 512), dtype=fp8)
    qmzc[:, 512:] = Vm.astype(fp8)
    # bk is a per-query constant score shift (softmax-invariant) -> dropped;
    # bov shifts O by a constant vector -> folded into the wobo bias row
    bo_eff = bo + Wo @ bov
    return {
        "xh": _xh(x[b], _key_order(s)),
        "blobw": _blobw(bq),
        "blob1": _blob1(Wq, Wk, Wov),
        "umask": np.ascontiguousarray(U.astype(fp8)),
        "qmzc": np.ascontiguousarray(qmzc),
        "qz": np.zeros((64, 2048), dtype=fp8),
        "wobo": np.concatenate([Wo.T / 32.0, bo_eff[None, :]], axis=0).astype(
            ml_dtypes.bfloat16
        ),
    }


# ---------------------------------------------------------------- entry point


def kernel(x, Wq, bq, Wk, bk, Wov, bov, Wo, bo, _trace=False):
    from concourse import bass_utils

    x = np.ascontiguousarray(np.asarray(x, dtype=np.float32))
    args = [np.asarray(a, dtype=np.float32) for a in (Wq, bq, Wk, bk, Wov, bov, Wo, bo)]

    nc = _get_prog()
    in_maps = [_in_map(x, *args, core) for core in range(8)]

    res = bass_utils.run_bass_kernel_spmd(
        nc, in_maps, core_ids=list(range(8)), trace=_trace
    )

    y = np.empty((B, N, D), dtype=np.float32)
    for core in range(8):
        b, s = divmod(core, 2)
        yc = np.asarray(res.results[core]["y"]).astype(np.float32)
        y[b].reshape(64, CH, D)[s::2] = yc.reshape(32, CH, D)
    return y

